# revision 14
# baseline (speedup 1.0000x reference)
"""Chunk-based multi-head attention TRN2 kernel (8-core SPMD), bf16.

Full model: x[S,B,E] -> in_proj -> 16-head attention with block-causal
64-chunk mask -> out_proj.  Sharding: B(2) x head-groups(4) over 8 cores;
each core computes 4 heads of one batch and a partial out_proj, reduced
on host.

All device data is bf16 (PSUM accumulation stays fp32): halves HBM
traffic and keeps every matmul at 1 cycle/row.  Host-side layouts are
partition-major so every DMA moves fat (4-8KB) contiguous per-partition
segments.  Scores are computed transposed (scoresT[t,s]) with 64-row
stationaries via PE tile_position (no zero padding for the D=64
contraction).  The softmax denominator rides as a 65th row of the PV
matmul (ones column in vaug); its reciprocal is computed per head
directly from the praw copy and broadcast across partitions with a
DRAM-bounce stride-0 DMA, so each head's normalize chain overlaps the
next head's matmuls.  Scores for two key chunks share one [128,1024]
PSUM pair so one exp activation covers two chunks.  out_proj(c) is
interleaved between in_proj(c+1) and attention(c+1); y tiles for a
chunk batch into one DMA.
"""

import sys

if "/opt/trn_rl_repo" not in sys.path:
    sys.path.insert(0, "/opt/trn_rl_repo")

import numpy as np
import ml_dtypes

import concourse.bass as bass
import concourse.mybir as mybir
import concourse.tile as tile
from concourse import bacc, bass_utils

S = 2048          # sequence length
B = 2             # batch
E = 1024          # embed dim
H = 16            # total heads
HL = 4            # heads per core
D = 64            # head dim
FQK = 2 * HL * D  # local q+k features = 512
FV = HL * D       # local v features = 256
KT = E // 128     # 8 contraction chunks for projections
NC = S // 512     # 4 query 512-chunks
TT = S // 128     # 16 key 128-chunks
N_CORES = 8

F32 = mybir.dt.float32
F32R = mybir.dt.float32r
BF16 = mybir.dt.bfloat16
BF16_NP = ml_dtypes.bfloat16

TRACE = False     # set by test.py for profiling runs
LAST_RESULT = None

_NC_CACHE = {}


def _body(nc, tc, x_d, wqk_d, wv_d, bqk_d, wo_d, y_d):
    from contextlib import ExitStack

    ctx = ExitStack()
    with ctx:
        P = ctx.enter_context(tc.tile_pool(name="persist", bufs=1))
        ep = ctx.enter_context(tc.tile_pool(name="etiles", bufs=1))
        np_ = ctx.enter_context(tc.tile_pool(name="normpool", bufs=1))
        xp = ctx.enter_context(tc.tile_pool(name="xstream", bufs=2))
        yp = ctx.enter_context(tc.tile_pool(name="ytiles", bufs=1))
        psum = ctx.enter_context(tc.tile_pool(name="psum", bufs=1, space="PSUM"))
        dp = ctx.enter_context(tc.tile_pool(name="dscratch", bufs=1, space="DRAM"))

        # ---- persistent SBUF ----
        qkT = P.tile([128, 2, S], BF16)       # qT: head 2kk on p0-63, 2kk+1 on p64-127
        kTz = P.tile([128, HL, S], BF16)      # per-head kT, off-parity half zeroed
        outT = P.tile([128, 2, S], BF16)      # normalized attn out, transposed
        vaug = P.tile([128, TT, HL, 65], BF16)  # per (T,h): [V_h(64) | ones]
        wqk_sb = P.tile([128, KT, FQK], BF16)
        wv_sb = P.tile([128, KT, FV], BF16)
        wo_sb = P.tile([128, 2, E], BF16)
        bqk_sb = P.tile([128, 4], F32)
        ones65 = P.tile([65, 128], F32)       # K=1@p64 stationary for denom bcast
        drec_dram = dp.tile([NC, HL, 512], F32)

        # chunk-0 x (split so the first matmuls start early) and weights
        xq0 = xp.tile([128, KT, 512], BF16, tag="xq")
        nc.sync.dma_start(out=xq0[:, 0:2, :], in_=x_d[:, 0, 0:2, :])
        nc.sync.dma_start(out=wqk_sb[:, 0:2, :], in_=wqk_d[:, 0:2, :])
        nc.sync.dma_start(out=xq0[:, 2:4, :], in_=x_d[:, 0, 2:4, :])
        nc.sync.dma_start(out=wqk_sb[:, 2:4, :], in_=wqk_d[:, 2:4, :])
        nc.sync.dma_start(out=xq0[:, 4:KT, :], in_=x_d[:, 0, 4:KT, :])
        nc.sync.dma_start(out=wqk_sb[:, 4:KT, :], in_=wqk_d[:, 4:KT, :])
        nc.sync.dma_start(out=wv_sb, in_=wv_d)
        nc.sync.dma_start(out=bqk_sb, in_=bqk_d.rearrange("(m p) -> p m", p=128))
        nc.vector.memset(ones65, 1.0)
        for h in range(HL):
            zpo = 64 if h % 2 == 0 else 0
            nc.vector.memset(kTz[zpo:zpo + 64, h, :], 0.0)
        nc.vector.memset(vaug[:, :, :, 64:65], 1.0)

        def in_proj_chunk0_kmajor(xq):
            # chunk 0 streams with the initial x DMA: iterate k outer with two
            # concurrent PSUM accumulators so the PE starts on the first slice
            ncols = slice(0, 512)
            for mp in (0, 2):
                ps0 = psum.tile([128, 512], F32, tag="io", bufs=2)
                ps1 = psum.tile([128, 512], F32, tag="io", bufs=2)
                for k in range(KT):
                    nc.tensor.matmul(
                        ps0, wqk_sb[:, k, mp * 128:(mp + 1) * 128], xq[:, k, :],
                        start=(k == 0), stop=(k == KT - 1))
                    nc.tensor.matmul(
                        ps1, wqk_sb[:, k, (mp + 1) * 128:(mp + 2) * 128], xq[:, k, :],
                        start=(k == 0), stop=(k == KT - 1))
                for m, ps in ((mp, ps0), (mp + 1, ps1)):
                    if m < 2:
                        nc.vector.tensor_scalar_add(
                            qkT[:, m, ncols], ps, bqk_sb[:, m:m + 1])
                    else:
                        h0, h1 = 2 * (m - 2), 2 * (m - 2) + 1
                        nc.vector.tensor_scalar_add(
                            kTz[0:64, h0, ncols], ps[0:64, :], bqk_sb[0:64, m:m + 1])
                        nc.vector.tensor_scalar_add(
                            kTz[64:128, h1, ncols], ps[64:128, :], bqk_sb[64:128, m:m + 1])
            for tp in (0, 2):
                ps0 = psum.tile([128, 512], F32, tag="io", bufs=2)
                ps1 = psum.tile([128, 512], F32, tag="io", bufs=2)
                for k in range(KT):
                    nc.tensor.matmul(
                        ps0[:, 0:FV], xq[:, k, tp * 128:(tp + 1) * 128], wv_sb[:, k, :],
                        start=(k == 0), stop=(k == KT - 1))
                    nc.tensor.matmul(
                        ps1[:, 0:FV], xq[:, k, (tp + 1) * 128:(tp + 2) * 128], wv_sb[:, k, :],
                        start=(k == 0), stop=(k == KT - 1))
                for tt, ps in ((tp, ps0), (tp + 1, ps1)):
                    nc.vector.tensor_copy(
                        vaug[:, tt, :, 0:64],
                        ps[:, 0:FV].rearrange("p (h d) -> p h d", h=HL))

        def in_proj_chunk(n, xq):
            ncols = slice(n * 512, (n + 1) * 512)
            for m in range(4):
                ps = psum.tile([128, 512], F32, tag="io", bufs=2)
                for k in range(KT):
                    nc.tensor.matmul(
                        ps,
                        wqk_sb[:, k, m * 128:(m + 1) * 128],
                        xq[:, k, :],
                        start=(k == 0), stop=(k == KT - 1),
                    )
                if m < 2:
                    nc.vector.tensor_scalar_add(qkT[:, m, ncols], ps, bqk_sb[:, m:m + 1])
                else:
                    h0, h1 = 2 * (m - 2), 2 * (m - 2) + 1
                    nc.vector.tensor_scalar_add(
                        kTz[0:64, h0, ncols], ps[0:64, :], bqk_sb[0:64, m:m + 1])
                    nc.vector.tensor_scalar_add(
                        kTz[64:128, h1, ncols], ps[64:128, :], bqk_sb[64:128, m:m + 1])
            for tt in range(4):
                t = 4 * n + tt
                ps = psum.tile([128, 512], F32, tag="io", bufs=2)
                for k in range(KT):
                    nc.tensor.matmul(
                        ps[:, 0:FV],
                        xq[:, k, tt * 128:(tt + 1) * 128],
                        wv_sb[:, k, :],
                        start=(k == 0), stop=(k == KT - 1),
                    )
                nc.vector.tensor_copy(
                    vaug[:, t, :, 0:64],
                    ps[:, 0:FV].rearrange("p (h d) -> p h d", h=HL),
                )

        HEAD_ORDER = [1, 3, 0, 2]  # odd-parity (staged-DMA) heads first

        def attention_chunk(c):
            # pv rows 0-63 = unnormalized attn-out (transposed), row 64 =
            # softmax denom.  Key chunks are paired: both scores land in one
            # [128,1024] PSUM pair so one exp covers two chunks.  Each head
            # normalizes immediately (per-head reciprocal + DRAM-bounce
            # broadcast) so the chain hides under the next head's matmuls.
            t_max = 4 * c + 3
            qcol0 = c * 512
            ccols = slice(c * 512, (c + 1) * 512)
            for h in HEAD_ORDER:
                kk = h // 2
                po = (h % 2) * 64
                pv = psum.tile([65, 512], F32, tag="pv", bufs=2)

                def emit_pair(Ta):
                    Tb = Ta + 1
                    s0a = max(0, (2 * Ta - 8 * c) * 64)
                    s0b = max(0, (2 * Tb - 8 * c) * 64)
                    sc2 = psum.tile([128, 1024], F32, tag="sc2", bufs=2)
                    nc.tensor.matmul(
                        sc2[:, s0a:512],
                        kTz[:, h, Ta * 128:(Ta + 1) * 128],
                        qkT[:, kk, qcol0 + s0a:qcol0 + 512],
                        start=True, stop=True,
                    )
                    nc.tensor.matmul(
                        sc2[:, 512 + s0b:1024],
                        kTz[:, h, Tb * 128:(Tb + 1) * 128],
                        qkT[:, kk, qcol0 + s0b:qcol0 + 512],
                        start=True, stop=True,
                    )
                    e2 = ep.tile([128, 1024], BF16, tag="e", bufs=3)
                    if s0b == 0:
                        nc.scalar.activation(
                            e2[:, s0a:1024], sc2[:, s0a:1024],
                            mybir.ActivationFunctionType.Exp, scale=0.125,
                        )
                    else:
                        # staircase pair: the [512, 512+s0b) gap is unwritten
                        nc.scalar.activation(
                            e2[:, s0a:512], sc2[:, s0a:512],
                            mybir.ActivationFunctionType.Exp, scale=0.125,
                        )
                        nc.scalar.activation(
                            e2[:, 512 + s0b:1024], sc2[:, 512 + s0b:1024],
                            mybir.ActivationFunctionType.Exp, scale=0.125,
                        )
                    # staircase: key chunk 2T+1 masked for first 64 query cols
                    if 2 * Ta - 8 * c >= 0:
                        nc.gpsimd.memset(e2[64:128, s0a:s0a + 64], 0.0)
                    if 2 * Tb - 8 * c >= 0:
                        nc.gpsimd.memset(e2[64:128, 512 + s0b:512 + s0b + 64], 0.0)
                    return e2, s0a, s0b, Ta, Tb

                # software pipeline: scores(pair i+1) issue on PE before
                # PV(pair i) so the ACT exp latency hides under PE work
                pend = emit_pair(0)
                for Ta in range(0, t_max + 1, 2):
                    nxt = emit_pair(Ta + 2) if Ta + 2 <= t_max else None
                    e2, s0a, s0b, Ta_, Tb = pend
                    nc.tensor.matmul(
                        pv[:, s0a:512],
                        vaug[:, Ta_, h, :],
                        e2[:, s0a:512],
                        start=(Ta_ == 0), stop=False,
                    )
                    nc.tensor.matmul(
                        pv[:, s0b:512],
                        vaug[:, Tb, h, :],
                        e2[:, 512 + s0b:1024],
                        start=False, stop=(Tb == t_max),
                    )
                    pend = nxt
                # per-head normalize chain: raw denom row -> DRAM -> stride-0
                # broadcast to 64 partitions -> reciprocal at base 0 -> mul.
                # (reciprocal_approx_fast is wrong at partition base 64.)
                praw = np_.tile([65, 512], F32, tag=f"praw{h}", bufs=2)
                nc.vector.tensor_copy(praw, pv)
                rbc = np_.tile([64, 512], F32, tag=f"rbc{h}", bufs=2)
                if c == NC - 1 and h == HEAD_ORDER[-1]:
                    # tail chain: broadcast the denom row across partitions
                    # with a K=1 ones matmul (no DRAM round trips)
                    ps_bc = psum.tile([128, 512], F32, tag="io", bufs=2)
                    nc.tensor.matmul(
                        ps_bc,
                        ones65[64:65, :],
                        praw[64:65, :],
                        start=True, stop=True,
                    )
                    nc.vector.reciprocal_approx_fast(out=rbc, in_=ps_bc[0:64, :])
                else:
                    nc.sync.dma_start(out=drec_dram[c, h], in_=praw[64:65, :])
                    rsrc = drec_dram[c, h]
                    bc_ap = bass.AP(tensor=rsrc.tensor, offset=rsrc.offset,
                                    ap=[[0, 64]] + [list(x) for x in rsrc.ap])
                    dbc = np_.tile([64, 512], F32, tag=f"dbc{h}", bufs=2)
                    nc.gpsimd.dma_start(out=dbc, in_=bc_ap)
                    nc.vector.reciprocal_approx_fast(out=rbc, in_=dbc)
                if po == 0:
                    nc.vector.tensor_mul(outT[0:64, kk, ccols], praw[0:64, :], rbc)
                else:
                    stage = np_.tile([64, 512], BF16, tag=f"stg{h}", bufs=2)
                    nc.vector.tensor_mul(stage, praw[0:64, :], rbc)
                    nc.sync.dma_start(out=outT[64:128, kk, ccols], in_=stage)

        def out_proj_chunk(c):
            # y tiles for chunk c batch into ybuf, then one fat DMA
            ybuf = yp.tile([128, 4, 1024], BF16, tag="ysb", bufs=2)
            for tt in range(4):
                t = 4 * c + tt
                for n in range(2):
                    ps_y = psum.tile([128, 512], F32, tag="io", bufs=2)
                    for kk in range(2):
                        nc.tensor.matmul(
                            ps_y,
                            outT[:, kk, t * 128:(t + 1) * 128],
                            wo_sb[:, kk, n * 512:(n + 1) * 512],
                            start=(kk == 0), stop=(kk == 1),
                        )
                    dst = ybuf[:, tt, n * 512:(n + 1) * 512]
                    if c == NC - 1 and (2 * tt + n) % 2 == 1:
                        nc.scalar.copy(dst, ps_y)  # ACT engine is idle post-exp
                    else:
                        nc.vector.tensor_copy(dst, ps_y)
                if c == NC - 1:
                    eng = nc.gpsimd if tt % 2 == 0 else nc.sync
                    eng.dma_start(out=y_d[c][:, tt, :], in_=ybuf[:, tt, :])
            if c < NC - 1:
                nc.gpsimd.dma_start(out=y_d[c], in_=ybuf)

        # ---- interleaved pipeline ----
        xq = xq0
        for c in range(NC):
            if c == 0:
                in_proj_chunk0_kmajor(xq)
            else:
                in_proj_chunk(c, xq)
            if c > 0:
                out_proj_chunk(c - 1)
            if c + 1 < NC:
                xq = xp.tile([128, KT, 512], BF16, tag="xq")
                nc.sync.dma_start(out=xq, in_=x_d[:, c + 1, :, :])
            if c == 0:
                nc.sync.dma_start(
                    out=wo_sb, in_=wo_d.rearrange("(kk p) f -> p kk f", p=128))
            attention_chunk(c)
        out_proj_chunk(NC - 1)


def build_program():
    key = "prog_bf16"
    if key in _NC_CACHE:
        return _NC_CACHE[key]
    nc = bacc.Bacc(
        "TRN2",
        target_bir_lowering=False,
        debug=False,
        enable_asserts=False,
        num_devices=N_CORES,
    )
    x_d = nc.dram_tensor("xblk", [128, NC, KT, 512], BF16, kind="ExternalInput").ap()
    wqk_d = nc.dram_tensor("wqkblk", [128, KT, FQK], BF16, kind="ExternalInput").ap()
    wv_d = nc.dram_tensor("wvblk", [128, KT, FV], BF16, kind="ExternalInput").ap()
    bqk_d = nc.dram_tensor("bqk", [FQK], F32, kind="ExternalInput").ap()
    wo_d = nc.dram_tensor("woT", [FV, E], BF16, kind="ExternalInput").ap()
    y_d = nc.dram_tensor("y", [NC, 128, 4, 1024], BF16, kind="ExternalOutput").ap()

    with tile.TileContext(nc) as tc:
        _body(nc, tc, x_d, wqk_d, wv_d, bqk_d, wo_d, y_d)
    nc.compile()
    _NC_CACHE[key] = nc
    return nc


def make_in_maps(x, in_proj_w, in_proj_b, out_proj_w):
    x = np.asarray(x, dtype=np.float32)
    W = np.asarray(in_proj_w, dtype=np.float32)
    bi = np.asarray(in_proj_b, dtype=np.float32)
    Wo = np.asarray(out_proj_w, dtype=np.float32)
    in_maps = []
    for core in range(N_CORES):
        b = core // 4
        g = core % 4
        qs = slice(g * FV, (g + 1) * FV)
        ks = slice(E + g * FV, E + (g + 1) * FV)
        vs = slice(2 * E + g * FV, 2 * E + (g + 1) * FV)
        xT = np.ascontiguousarray(x[:, b, :].T)               # [E, S]
        # [k, p, n, f] -> [p, n, k, f] so per-partition segments are 8KB
        xblk = xT.reshape(KT, 128, NC, 512).transpose(1, 2, 0, 3)
        wqkT = np.concatenate([W[qs], W[ks]], axis=0).T        # [E, FQK]
        wqkblk = wqkT.reshape(KT, 128, FQK).transpose(1, 0, 2)
        wvblk = W[vs].T.reshape(KT, 128, FV).transpose(1, 0, 2)
        in_maps.append({
            "xblk": np.ascontiguousarray(xblk).astype(BF16_NP),
            "wqkblk": np.ascontiguousarray(wqkblk).astype(BF16_NP),
            "wvblk": np.ascontiguousarray(wvblk).astype(BF16_NP),
            "bqk": np.ascontiguousarray(np.concatenate([bi[qs], bi[ks]])),
            "woT": np.ascontiguousarray(Wo[:, g * FV:(g + 1) * FV].T).astype(BF16_NP),
        })
    return in_maps


def _unblock_y(yb):
    # [NC, 128, 4, 1024] -> [S, E]:  y[c*512 + tt*128 + p, e] = yb[c, p, tt, e]
    return np.transpose(np.asarray(yb), (0, 2, 1, 3)).reshape(S, E)


def kernel(x, in_proj_w, in_proj_b, out_proj_w, out_proj_b):
    global LAST_RESULT
    nc = build_program()
    in_maps = make_in_maps(x, in_proj_w, in_proj_b, out_proj_w)
    res = bass_utils.run_bass_kernel_spmd(
        nc, in_maps, core_ids=list(range(N_CORES)), trace=TRACE,
    )
    LAST_RESULT = res
    bo = np.asarray(out_proj_b, dtype=np.float32)
    bi = np.asarray(in_proj_b, dtype=np.float32)
    bo = bo + np.asarray(out_proj_w, dtype=np.float32) @ bi[2 * E:3 * E]
    out = np.zeros((S, B, E), dtype=np.float32)
    for b in range(B):
        acc = _unblock_y(res.results[b * 4]["y"]).astype(np.float32)
        for g in range(1, 4):
            acc = acc + _unblock_y(res.results[b * 4 + g]["y"]).astype(np.float32)
        out[:, b, :] = acc + bo[None, :]
    return out


# revision 15
# speedup vs baseline: 1.0221x; 1.0221x over previous
"""Chunk-based multi-head attention TRN2 kernel (8-core SPMD), bf16.

Full model: x[S,B,E] -> in_proj -> 16-head attention with block-causal
64-chunk mask -> out_proj.  Sharding: B(2) x head-groups(4) over 8 cores;
each core computes 4 heads of one batch and a partial out_proj, reduced
on host.

All device data is bf16 (PSUM accumulation stays fp32): halves HBM
traffic and keeps every matmul at 1 cycle/row.  Host-side layouts are
partition-major so every DMA moves fat (4-8KB) contiguous per-partition
segments.  Scores are computed transposed (scoresT[t,s]) with 64-row
stationaries via PE tile_position (no zero padding for the D=64
contraction).  The softmax denominator rides as a 65th row of the PV
matmul (ones column in vaug); its reciprocal is computed per head
directly from the praw copy and broadcast across partitions with a
DRAM-bounce stride-0 DMA, so each head's normalize chain overlaps the
next head's matmuls.  Scores for two key chunks share one [128,1024]
PSUM pair so one exp activation covers two chunks.  out_proj(c) is
interleaved between in_proj(c+1) and attention(c+1); y tiles for a
chunk batch into one DMA.
"""

import sys

if "/opt/trn_rl_repo" not in sys.path:
    sys.path.insert(0, "/opt/trn_rl_repo")

import numpy as np
import ml_dtypes

import concourse.bass as bass
import concourse.mybir as mybir
import concourse.tile as tile
from concourse import bacc, bass_utils

S = 2048          # sequence length
B = 2             # batch
E = 1024          # embed dim
H = 16            # total heads
HL = 4            # heads per core
D = 64            # head dim
FQK = 2 * HL * D  # local q+k features = 512
FV = HL * D       # local v features = 256
KT = E // 128     # 8 contraction chunks for projections
NC = S // 512     # 4 query 512-chunks
TT = S // 128     # 16 key 128-chunks
N_CORES = 8

F32 = mybir.dt.float32
F32R = mybir.dt.float32r
BF16 = mybir.dt.bfloat16
BF16_NP = ml_dtypes.bfloat16

TRACE = False     # set by test.py for profiling runs
LAST_RESULT = None

_NC_CACHE = {}


def _body(nc, tc, x_d, wqk_d, wv_d, bqk_d, wo_d, y_d):
    from contextlib import ExitStack

    ctx = ExitStack()
    with ctx:
        P = ctx.enter_context(tc.tile_pool(name="persist", bufs=1))
        ep = ctx.enter_context(tc.tile_pool(name="etiles", bufs=1))
        np_ = ctx.enter_context(tc.tile_pool(name="normpool", bufs=1))
        xp = ctx.enter_context(tc.tile_pool(name="xstream", bufs=2))
        yp = ctx.enter_context(tc.tile_pool(name="ytiles", bufs=1))
        psum = ctx.enter_context(tc.tile_pool(name="psum", bufs=1, space="PSUM"))
        dp = ctx.enter_context(tc.tile_pool(name="dscratch", bufs=1, space="DRAM"))

        # ---- persistent SBUF ----
        qkT = P.tile([128, 2, S], BF16)       # qT: head 2kk on p0-63, 2kk+1 on p64-127
        kTz = P.tile([128, HL, S], BF16)      # per-head kT, off-parity half zeroed
        outT = P.tile([128, 2, S], BF16)      # normalized attn out, transposed
        vaug = P.tile([128, TT, HL, 65], BF16)  # per (T,h): [V_h(64) | ones]
        wqk_sb = P.tile([128, KT, FQK], BF16)
        wv_sb = P.tile([128, KT, FV], BF16)
        wo_sb = P.tile([128, 2, E], BF16)
        bqk_sb = P.tile([128, 4], F32)
        ones65 = P.tile([65, 128], F32)       # K=1@p64 stationary for denom bcast
        drec_dram = dp.tile([NC, HL, 512], F32)

        # chunk-0 x (split so the first matmuls start early) and weights
        xq0 = xp.tile([128, KT, 512], BF16, tag="xq")
        nc.sync.dma_start(out=xq0[:, 0:2, :], in_=x_d[:, 0, 0:2, :])
        nc.sync.dma_start(out=wqk_sb[:, 0:2, :], in_=wqk_d[:, 0:2, :])
        nc.sync.dma_start(out=xq0[:, 2:4, :], in_=x_d[:, 0, 2:4, :])
        nc.sync.dma_start(out=wqk_sb[:, 2:4, :], in_=wqk_d[:, 2:4, :])
        nc.sync.dma_start(out=xq0[:, 4:KT, :], in_=x_d[:, 0, 4:KT, :])
        nc.sync.dma_start(out=wqk_sb[:, 4:KT, :], in_=wqk_d[:, 4:KT, :])
        nc.sync.dma_start(out=wv_sb, in_=wv_d)
        nc.sync.dma_start(out=bqk_sb, in_=bqk_d.rearrange("(m p) -> p m", p=128))
        nc.vector.memset(ones65, 1.0)
        for h in range(HL):
            zpo = 64 if h % 2 == 0 else 0
            nc.gpsimd.memset(kTz[zpo:zpo + 64, h, :], 0.0)
        nc.vector.memset(vaug[:, :, :, 64:65], 1.0)

        def in_proj_chunk0_kmajor(xq):
            # chunk 0 streams with the initial x DMA: iterate k outer with two
            # concurrent PSUM accumulators so the PE starts on the first slice
            ncols = slice(0, 512)
            for mp in (0, 2):
                ps0 = psum.tile([128, 512], F32, tag="io", bufs=2)
                ps1 = psum.tile([128, 512], F32, tag="io", bufs=2)
                for k in range(KT):
                    nc.tensor.matmul(
                        ps0, wqk_sb[:, k, mp * 128:(mp + 1) * 128], xq[:, k, :],
                        start=(k == 0), stop=(k == KT - 1))
                    nc.tensor.matmul(
                        ps1, wqk_sb[:, k, (mp + 1) * 128:(mp + 2) * 128], xq[:, k, :],
                        start=(k == 0), stop=(k == KT - 1))
                for m, ps in ((mp, ps0), (mp + 1, ps1)):
                    if m < 2:
                        nc.vector.tensor_scalar_add(
                            qkT[:, m, ncols], ps, bqk_sb[:, m:m + 1])
                    else:
                        h0, h1 = 2 * (m - 2), 2 * (m - 2) + 1
                        nc.vector.tensor_scalar_add(
                            kTz[0:64, h0, ncols], ps[0:64, :], bqk_sb[0:64, m:m + 1])
                        nc.vector.tensor_scalar_add(
                            kTz[64:128, h1, ncols], ps[64:128, :], bqk_sb[64:128, m:m + 1])
            for tp in (0, 2):
                psv = psum.tile([128, 1024], F32, tag="sc2", bufs=2)
                for k in range(KT):
                    nc.tensor.matmul(
                        psv[:, 0:FV], xq[:, k, tp * 128:(tp + 1) * 128], wv_sb[:, k, :],
                        start=(k == 0), stop=(k == KT - 1))
                    nc.tensor.matmul(
                        psv[:, 512:512 + FV], xq[:, k, (tp + 1) * 128:(tp + 2) * 128],
                        wv_sb[:, k, :],
                        start=(k == 0), stop=(k == KT - 1))
                for tt, off in ((tp, 0), (tp + 1, 512)):
                    nc.vector.tensor_copy(
                        vaug[:, tt, :, 0:64],
                        psv[:, off:off + FV].rearrange("p (h d) -> p h d", h=HL))

        def in_proj_chunk(n, xq):
            ncols = slice(n * 512, (n + 1) * 512)
            for m in range(4):
                ps = psum.tile([128, 512], F32, tag="io", bufs=2)
                for k in range(KT):
                    nc.tensor.matmul(
                        ps,
                        wqk_sb[:, k, m * 128:(m + 1) * 128],
                        xq[:, k, :],
                        start=(k == 0), stop=(k == KT - 1),
                    )
                if m < 2:
                    nc.vector.tensor_scalar_add(qkT[:, m, ncols], ps, bqk_sb[:, m:m + 1])
                else:
                    h0, h1 = 2 * (m - 2), 2 * (m - 2) + 1
                    nc.vector.tensor_scalar_add(
                        kTz[0:64, h0, ncols], ps[0:64, :], bqk_sb[0:64, m:m + 1])
                    nc.vector.tensor_scalar_add(
                        kTz[64:128, h1, ncols], ps[64:128, :], bqk_sb[64:128, m:m + 1])
            for tt in range(4):
                t = 4 * n + tt
                ps = psum.tile([128, 512], F32, tag="io", bufs=2)
                for k in range(KT):
                    nc.tensor.matmul(
                        ps[:, 0:FV],
                        xq[:, k, tt * 128:(tt + 1) * 128],
                        wv_sb[:, k, :],
                        start=(k == 0), stop=(k == KT - 1),
                    )
                nc.vector.tensor_copy(
                    vaug[:, t, :, 0:64],
                    ps[:, 0:FV].rearrange("p (h d) -> p h d", h=HL),
                )

        HEAD_ORDER = [1, 3, 0, 2]  # odd-parity (staged-DMA) heads first

        def attention_chunk(c, after_first_head=None):
            # pv rows 0-63 = unnormalized attn-out (transposed), row 64 =
            # softmax denom.  Key chunks are paired: both scores land in one
            # [128,1024] PSUM pair so one exp covers two chunks.  Each head
            # normalizes immediately (per-head reciprocal + DRAM-bounce
            # broadcast) so the chain hides under the next head's matmuls.
            t_max = 4 * c + 3
            qcol0 = c * 512
            ccols = slice(c * 512, (c + 1) * 512)
            for hi, h in enumerate(HEAD_ORDER):
                if hi == 1 and after_first_head is not None:
                    after_first_head()
                kk = h // 2
                po = (h % 2) * 64
                pv = psum.tile([65, 512], F32, tag="pv", bufs=2)

                def emit_pair(Ta):
                    Tb = Ta + 1
                    s0a = max(0, (2 * Ta - 8 * c) * 64)
                    s0b = max(0, (2 * Tb - 8 * c) * 64)
                    sc2 = psum.tile([128, 1024], F32, tag="sc2", bufs=2)
                    nc.tensor.matmul(
                        sc2[:, s0a:512],
                        kTz[:, h, Ta * 128:(Ta + 1) * 128],
                        qkT[:, kk, qcol0 + s0a:qcol0 + 512],
                        start=True, stop=True,
                    )
                    nc.tensor.matmul(
                        sc2[:, 512 + s0b:1024],
                        kTz[:, h, Tb * 128:(Tb + 1) * 128],
                        qkT[:, kk, qcol0 + s0b:qcol0 + 512],
                        start=True, stop=True,
                    )
                    e2 = ep.tile([128, 1024], BF16, tag="e", bufs=3)
                    if s0b == 0:
                        nc.scalar.activation(
                            e2[:, s0a:1024], sc2[:, s0a:1024],
                            mybir.ActivationFunctionType.Exp, scale=0.125,
                        )
                    else:
                        # staircase pair: the [512, 512+s0b) gap is unwritten
                        nc.scalar.activation(
                            e2[:, s0a:512], sc2[:, s0a:512],
                            mybir.ActivationFunctionType.Exp, scale=0.125,
                        )
                        nc.scalar.activation(
                            e2[:, 512 + s0b:1024], sc2[:, 512 + s0b:1024],
                            mybir.ActivationFunctionType.Exp, scale=0.125,
                        )
                    # staircase: key chunk 2T+1 masked for first 64 query cols
                    if 2 * Ta - 8 * c >= 0:
                        nc.gpsimd.memset(e2[64:128, s0a:s0a + 64], 0.0)
                    if 2 * Tb - 8 * c >= 0:
                        nc.gpsimd.memset(e2[64:128, 512 + s0b:512 + s0b + 64], 0.0)
                    return e2, s0a, s0b, Ta, Tb

                # software pipeline: scores(pair i+1) issue on PE before
                # PV(pair i) so the ACT exp latency hides under PE work
                pend = emit_pair(0)
                for Ta in range(0, t_max + 1, 2):
                    nxt = emit_pair(Ta + 2) if Ta + 2 <= t_max else None
                    e2, s0a, s0b, Ta_, Tb = pend
                    nc.tensor.matmul(
                        pv[:, s0a:512],
                        vaug[:, Ta_, h, :],
                        e2[:, s0a:512],
                        start=(Ta_ == 0), stop=False,
                    )
                    nc.tensor.matmul(
                        pv[:, s0b:512],
                        vaug[:, Tb, h, :],
                        e2[:, 512 + s0b:1024],
                        start=False, stop=(Tb == t_max),
                    )
                    pend = nxt
                # per-head normalize chain: raw denom row -> DRAM -> stride-0
                # broadcast to 64 partitions -> reciprocal at base 0 -> mul.
                # (reciprocal_approx_fast is wrong at partition base 64.)
                praw = np_.tile([65, 512], F32, tag=f"praw{h}", bufs=2)
                nc.vector.tensor_copy(praw, pv)
                rbc = np_.tile([64, 512], F32, tag=f"rbc{h}", bufs=2)
                if c == NC - 1 and h == HEAD_ORDER[-1]:
                    # tail chain: broadcast the denom row across partitions
                    # with a K=1 ones matmul (no DRAM round trips)
                    ps_bc = psum.tile([128, 512], F32, tag="io", bufs=2)
                    nc.tensor.matmul(
                        ps_bc,
                        ones65[64:65, :],
                        praw[64:65, :],
                        start=True, stop=True,
                    )
                    nc.vector.reciprocal_approx_fast(out=rbc, in_=ps_bc[0:64, :])
                else:
                    nc.sync.dma_start(out=drec_dram[c, h], in_=praw[64:65, :])
                    rsrc = drec_dram[c, h]
                    bc_ap = bass.AP(tensor=rsrc.tensor, offset=rsrc.offset,
                                    ap=[[0, 64]] + [list(x) for x in rsrc.ap])
                    dbc = np_.tile([64, 512], F32, tag=f"dbc{h}", bufs=2)
                    nc.gpsimd.dma_start(out=dbc, in_=bc_ap)
                    nc.vector.reciprocal_approx_fast(out=rbc, in_=dbc)
                if po == 0:
                    nc.vector.tensor_mul(outT[0:64, kk, ccols], praw[0:64, :], rbc)
                else:
                    stage = np_.tile([64, 512], BF16, tag=f"stg{h}", bufs=2)
                    nc.vector.tensor_mul(stage, praw[0:64, :], rbc)
                    nc.sync.dma_start(out=outT[64:128, kk, ccols], in_=stage)

        def out_proj_chunk(c):
            # y tiles for chunk c batch into ybuf, then one fat DMA
            ybuf = yp.tile([128, 4, 1024], BF16, tag="ysb", bufs=2)
            for tt in range(4):
                t = 4 * c + tt
                for n in range(2):
                    ps_y = psum.tile([128, 512], F32, tag="io", bufs=2)
                    for kk in range(2):
                        nc.tensor.matmul(
                            ps_y,
                            outT[:, kk, t * 128:(t + 1) * 128],
                            wo_sb[:, kk, n * 512:(n + 1) * 512],
                            start=(kk == 0), stop=(kk == 1),
                        )
                    dst = ybuf[:, tt, n * 512:(n + 1) * 512]
                    if c == NC - 1 and (2 * tt + n) % 2 == 1:
                        nc.scalar.copy(dst, ps_y)  # ACT engine is idle post-exp
                    else:
                        nc.vector.tensor_copy(dst, ps_y)
                if c == NC - 1:
                    eng = nc.gpsimd if tt % 2 == 0 else nc.sync
                    eng.dma_start(out=y_d[c][:, tt, :], in_=ybuf[:, tt, :])
            if c < NC - 1:
                nc.gpsimd.dma_start(out=y_d[c], in_=ybuf)

        # ---- interleaved pipeline ----
        xq = xq0
        for c in range(NC):
            if c == 0:
                in_proj_chunk0_kmajor(xq)
            else:
                in_proj_chunk(c, xq)
            if c + 1 < NC:
                xq = xp.tile([128, KT, 512], BF16, tag="xq")
                nc.sync.dma_start(out=xq, in_=x_d[:, c + 1, :, :])
            if c == 0:
                nc.sync.dma_start(
                    out=wo_sb, in_=wo_d.rearrange("(kk p) f -> p kk f", p=128))
            cb = (lambda cc=c: out_proj_chunk(cc - 1)) if c > 0 else None
            attention_chunk(c, after_first_head=cb)
        out_proj_chunk(NC - 1)


def build_program():
    key = "prog_bf16"
    if key in _NC_CACHE:
        return _NC_CACHE[key]
    nc = bacc.Bacc(
        "TRN2",
        target_bir_lowering=False,
        debug=False,
        enable_asserts=False,
        num_devices=N_CORES,
    )
    x_d = nc.dram_tensor("xblk", [128, NC, KT, 512], BF16, kind="ExternalInput").ap()
    wqk_d = nc.dram_tensor("wqkblk", [128, KT, FQK], BF16, kind="ExternalInput").ap()
    wv_d = nc.dram_tensor("wvblk", [128, KT, FV], BF16, kind="ExternalInput").ap()
    bqk_d = nc.dram_tensor("bqk", [FQK], F32, kind="ExternalInput").ap()
    wo_d = nc.dram_tensor("woT", [FV, E], BF16, kind="ExternalInput").ap()
    y_d = nc.dram_tensor("y", [NC, 128, 4, 1024], BF16, kind="ExternalOutput").ap()

    with tile.TileContext(nc) as tc:
        _body(nc, tc, x_d, wqk_d, wv_d, bqk_d, wo_d, y_d)
    nc.compile()
    _NC_CACHE[key] = nc
    return nc


def make_in_maps(x, in_proj_w, in_proj_b, out_proj_w):
    x = np.asarray(x, dtype=np.float32)
    W = np.asarray(in_proj_w, dtype=np.float32)
    bi = np.asarray(in_proj_b, dtype=np.float32)
    Wo = np.asarray(out_proj_w, dtype=np.float32)
    in_maps = []
    for core in range(N_CORES):
        b = core // 4
        g = core % 4
        qs = slice(g * FV, (g + 1) * FV)
        ks = slice(E + g * FV, E + (g + 1) * FV)
        vs = slice(2 * E + g * FV, 2 * E + (g + 1) * FV)
        xT = np.ascontiguousarray(x[:, b, :].T)               # [E, S]
        # [k, p, n, f] -> [p, n, k, f] so per-partition segments are 8KB
        xblk = xT.reshape(KT, 128, NC, 512).transpose(1, 2, 0, 3)
        wqkT = np.concatenate([W[qs], W[ks]], axis=0).T        # [E, FQK]
        wqkblk = wqkT.reshape(KT, 128, FQK).transpose(1, 0, 2)
        wvblk = W[vs].T.reshape(KT, 128, FV).transpose(1, 0, 2)
        in_maps.append({
            "xblk": np.ascontiguousarray(xblk).astype(BF16_NP),
            "wqkblk": np.ascontiguousarray(wqkblk).astype(BF16_NP),
            "wvblk": np.ascontiguousarray(wvblk).astype(BF16_NP),
            "bqk": np.ascontiguousarray(np.concatenate([bi[qs], bi[ks]])),
            "woT": np.ascontiguousarray(Wo[:, g * FV:(g + 1) * FV].T).astype(BF16_NP),
        })
    return in_maps


def _unblock_y(yb):
    # [NC, 128, 4, 1024] -> [S, E]:  y[c*512 + tt*128 + p, e] = yb[c, p, tt, e]
    return np.transpose(np.asarray(yb), (0, 2, 1, 3)).reshape(S, E)


def kernel(x, in_proj_w, in_proj_b, out_proj_w, out_proj_b):
    global LAST_RESULT
    nc = build_program()
    in_maps = make_in_maps(x, in_proj_w, in_proj_b, out_proj_w)
    res = bass_utils.run_bass_kernel_spmd(
        nc, in_maps, core_ids=list(range(N_CORES)), trace=TRACE,
    )
    LAST_RESULT = res
    bo = np.asarray(out_proj_b, dtype=np.float32)
    bi = np.asarray(in_proj_b, dtype=np.float32)
    bo = bo + np.asarray(out_proj_w, dtype=np.float32) @ bi[2 * E:3 * E]
    out = np.zeros((S, B, E), dtype=np.float32)
    for b in range(B):
        acc = _unblock_y(res.results[b * 4]["y"]).astype(np.float32)
        for g in range(1, 4):
            acc = acc + _unblock_y(res.results[b * 4 + g]["y"]).astype(np.float32)
        out[:, b, :] = acc + bo[None, :]
    return out
